# revision 1
# baseline (speedup 1.0000x reference)
"""Trainium2 Bass kernel for 3-layer GraphSAGE (mean agg) + global mean pool + linear head.

Sharding: nodes (and incident edges, by dst) are partitioned across 8 NeuronCores in
contiguous ranges.  Each SAGE layer:
  - gathers neighbor features h[src] from a replicated bf16 node-feature table in HBM
    (dma_gather, edge-major tiles),
  - scatter-means them into per-dst-window PSUM accumulators via one-hot
    selection-matrix matmuls on TensorE (S built on VectorE with iota+is_equal),
  - applies lin_l/lin_r (feature-major matmuls) + bias + ReLU on TensorE/ScalarE,
  - AllGathers the new node-feature shard into every core's table (chunked into
    sub-shards so next-layer gathers can start before the whole exchange finishes).
Segment-mean pooling over the (sorted) batch vector and the linear head run locally
per core; the host sums the per-core partial graph outputs.

Self-contained: hardcodes the problem shapes from the spec.
"""

import math
from dataclasses import dataclass

import numpy as np


# ----------------------------------------------------------------------------- config

@dataclass
class Cfg:
    n_nodes: int = 100000
    n_edges: int = 1600000
    n_graphs: int = 512
    n_cores: int = 8
    feat: int = 128           # = hidden
    n_cls: int = 32
    n_sub: int = 4            # sub-shards per core = gather chunks (int16 idx limit)
    gcall_wins: int = 7       # dst windows per gather call group

    @property
    def percore(self) -> int:
        return self.n_nodes // self.n_cores

    @property
    def n_win(self) -> int:
        return math.ceil(self.percore / 128)

    @property
    def wins_per_sub(self) -> int:
        return math.ceil(self.n_win / self.n_sub)

    def sub_rows(self, s: int) -> int:
        w0 = s * self.wins_per_sub
        w1 = min((s + 1) * self.wins_per_sub, self.n_win)
        return min(w1 * 128, self.percore) - w0 * 128

    @property
    def chunk_sizes(self) -> list[int]:
        return [self.sub_rows(s) * self.n_cores for s in range(self.n_sub)]

    @property
    def chunk_offs(self) -> list[int]:
        offs, o = [], 0
        for sz in self.chunk_sizes:
            offs.append(o)
            o += sz
        return offs

    @property
    def n_groups(self) -> int:
        return math.ceil(self.n_win / self.gcall_wins)

    def group_wins(self, g: int) -> tuple[int, int]:
        return g * self.gcall_wins, min((g + 1) * self.gcall_wins, self.n_win)


FULL = Cfg()


# ------------------------------------------------------------------- host-side prep

def _table_row(cfg: Cfg, n: np.ndarray) -> np.ndarray:
    """Global node id -> row in the (sub-shard-major) gather table."""
    c = n // cfg.percore
    i = n % cfg.percore
    s = np.minimum(i // (cfg.wins_per_sub * 128), cfg.n_sub - 1)
    j = i - s * (cfg.wins_per_sub * 128)
    chunk_offs = np.asarray(cfg.chunk_offs)
    sub_rows = np.asarray([cfg.sub_rows(k) for k in range(cfg.n_sub)])
    return chunk_offs[s] + c * sub_rows[s] + j


@dataclass
class ProgMeta:
    """Program-shape metadata — identical across cores (SPMD)."""
    bw: np.ndarray              # [n_win, n_sub] batches per (window, chunk)
    call_cbs: list              # [group][chunk] batches in that gather call
    batch_col: dict             # (w, k) -> first dstloc column of its batch run
    call_of: dict               # (g, k) -> (call_id, flat idx offset)
    nb_total: int
    gidx_total: int


def build_host_data(cfg: Cfg, x, edge_index, batch, params):
    import ml_dtypes

    bf16 = ml_dtypes.bfloat16
    src = np.asarray(edge_index[0], dtype=np.int64)
    dst = np.asarray(edge_index[1], dtype=np.int64)
    batch = np.asarray(batch, dtype=np.int64)
    x = np.asarray(x, dtype=np.float32)

    deg = np.bincount(dst, minlength=cfg.n_nodes).astype(np.float32)
    invdeg_all = 1.0 / np.maximum(deg, 1.0)
    cnt = np.bincount(batch, minlength=cfg.n_graphs).astype(np.float32)
    invcnt_all = 1.0 / np.maximum(cnt, 1.0)

    src_row = _table_row(cfg, src)
    chunk_offs = np.asarray(cfg.chunk_offs)
    src_chunk = np.searchsorted(chunk_offs, src_row, side="right") - 1
    src_local = src_row - chunk_offs[src_chunk]
    for k in range(cfg.n_sub):
        assert cfg.chunk_sizes[k] <= 32767, "int16 gather index overflow"

    core_of = dst // cfg.percore
    dst_local_all = dst % cfg.percore

    # per-core edge sets sorted by (window, chunk)
    per_core = []
    counts = np.zeros((cfg.n_cores, cfg.n_win, cfg.n_sub), dtype=np.int64)
    for c in range(cfg.n_cores):
        m = core_of == c
        dl = dst_local_all[m]
        w_arr = dl // 128
        k_arr = src_chunk[m]
        order = np.lexsort((k_arr, w_arr))
        e_sl = src_local[m][order].astype(np.int64)
        e_w = w_arr[order]
        e_k = k_arr[order]
        e_dl = (dl % 128).astype(np.float32)[order]
        per_core.append((e_sl, e_w, e_k, e_dl))
        np.add.at(counts[c], (e_w, e_k), 1)

    bw = np.ceil(counts.max(axis=0) / 128).astype(np.int64)
    no_batch = bw.sum(axis=1) == 0
    bw[no_batch, 0] = 1            # keep >=1 batch so PSUM is zeroed via start=True

    batch_col, col = {}, 0
    for w in range(cfg.n_win):
        for k in range(cfg.n_sub):
            batch_col[(w, k)] = col
            col += int(bw[w, k])
    nb_total = col

    call_cbs, call_of = [], {}
    call_id, gidx_off = 0, 0
    for g in range(cfg.n_groups):
        w0, w1 = cfg.group_wins(g)
        row = []
        for k in range(cfg.n_sub):
            cbs = int(bw[w0:w1, k].sum())
            row.append(cbs)
            if cbs > 0:
                call_of[(g, k)] = (call_id, gidx_off)
                gidx_off += cbs * 1024   # [128 partitions, cbs*8] int16 slots
                call_id += 1
        call_cbs.append(row)
    meta = ProgMeta(bw=bw, call_cbs=call_cbs, batch_col=batch_col,
                    call_of=call_of, nb_total=nb_total, gidx_total=gidx_off)

    # ---------------- shared (per-core-identical) tensors
    p_pad = cfg.n_win * 128
    trow_all = _table_row(cfg, np.arange(cfg.n_nodes))
    table0 = np.zeros((cfg.n_nodes, cfg.feat), dtype=bf16)
    table0[trow_all] = x.astype(bf16)

    w_fm = np.zeros((6, cfg.feat, cfg.feat), dtype=bf16)
    for l, (wl, wr) in enumerate(
            [(params["Wl0"], params["Wr0"]), (params["Wl1"], params["Wr1"]),
             (params["Wl2"], params["Wr2"])]):
        w_fm[2 * l] = np.asarray(wl, np.float32).T.astype(bf16)
        w_fm[2 * l + 1] = np.asarray(wr, np.float32).T.astype(bf16)
    bias_sb = np.stack([np.asarray(params[f"bl{l}"], np.float32)
                        for l in range(3)], axis=1)            # [128, 3]
    wlin_fm = np.asarray(params["Wlin"], np.float32).T.copy()  # [128, 32]

    iota = np.broadcast_to(np.arange(128, dtype=np.float32),
                           (128, 128)).astype(bf16).copy()
    ident_bf = np.eye(128, dtype=bf16)
    ident_f32 = np.eye(128, dtype=np.float32)

    in_maps, gbase_list = [], []
    for c in range(cfg.n_cores):
        e_sl, e_w, e_k, e_dl = per_core[c]
        gidx_flat = np.zeros(meta.gidx_total, dtype=np.int16)
        dstloc = np.full((128, nb_total), -1.0, dtype=np.float32)

        wk_keys = e_w * cfg.n_sub + e_k
        bounds = np.searchsorted(
            wk_keys, np.arange(cfg.n_win * cfg.n_sub + 1), side="left")
        for g in range(cfg.n_groups):
            w0, w1 = cfg.group_wins(g)
            for k in range(cfg.n_sub):
                if (g, k) not in call_of:
                    continue
                _, off0 = call_of[(g, k)]
                cbs_call = call_cbs[g][k]
                blkview = gidx_flat[off0:off0 + cbs_call * 1024].reshape(
                    128, cbs_call * 8)
                pos = 0
                for w in range(w0, w1):
                    lo, hi = bounds[w * cfg.n_sub + k], bounds[w * cfg.n_sub + k + 1]
                    nbat = int(bw[w, k])
                    if nbat == 0:
                        continue
                    n_e = hi - lo
                    sl = np.zeros(nbat * 128, dtype=np.int16)
                    dl_pad = np.full(nbat * 128, -1.0, dtype=np.float32)
                    sl[:n_e] = e_sl[lo:hi].astype(np.int16)
                    dl_pad[:n_e] = e_dl[lo:hi]
                    col0 = batch_col[(w, k)]
                    dstloc[:, col0:col0 + nbat] = dl_pad.reshape(nbat, 128).T
                    # idx packing: call position e -> [lane e%16, slot e//16],
                    # replicated over the 8 16-partition groups
                    blk = sl.reshape(nbat, 8, 16).transpose(0, 2, 1)  # [b,16,8]
                    for b in range(nbat):
                        blkview[:, (pos + b) * 8:(pos + b + 1) * 8] = np.tile(
                            blk[b], (8, 1))
                    pos += nbat

        own = np.arange(cfg.percore) + c * cfg.percore
        invd = np.ones((p_pad,), dtype=np.float32)
        invd[:cfg.percore] = invdeg_all[own]
        invdeg_t = invd.reshape(cfg.n_win, 128).T.copy()

        gb = int(batch[own[0]])
        gspan = int(batch[own[-1]]) - gb + 1
        assert gspan <= 128, f"graph span {gspan} > 128"
        gbase_list.append(gb)
        segt = np.zeros((cfg.n_win, 128, 128), dtype=np.float32)
        grel = (batch[own] - gb).astype(np.int64)
        ridx = np.arange(cfg.percore)
        segt[ridx // 128, ridx % 128, grel] = 1.0
        invcnt = np.ones((128, 1), dtype=np.float32)
        hi_g = min(gb + 128, cfg.n_graphs)
        invcnt[:hi_g - gb, 0] = invcnt_all[gb:hi_g]

        x_fm = np.zeros((cfg.feat, p_pad), dtype=bf16)
        x_fm[:, :cfg.percore] = x[own].T.astype(bf16)

        in_maps.append({
            "table0": table0, "x_fm": x_fm, "gidx": gidx_flat,
            "dstloc": dstloc, "invdeg": invdeg_t, "segt": segt,
            "invcnt": invcnt, "w_fm": w_fm, "bias_sb": bias_sb,
            "wlin_fm": wlin_fm, "iota": iota, "ident_bf": ident_bf,
            "ident_f32": ident_f32,
        })
    return in_maps, meta, gbase_list


# ------------------------------------------------------------------- device program

def build_program(cfg: Cfg, meta: ProgMeta, *, skip_gather=False,
                  skip_s_build=False, skip_ag=False):
    """skip_* flags produce wrong results; they exist only for perf attribution."""
    import concourse.bacc as bacc
    import concourse.tile as tile
    import concourse.mybir as mybir

    f32 = mybir.dt.float32
    bf = mybir.dt.bfloat16
    i16 = mybir.dt.int16
    F = cfg.feat
    p_pad = cfg.n_win * 128

    nc = bacc.Bacc("TRN2", target_bir_lowering=False, debug=False,
                   num_devices=cfg.n_cores)

    t_table0 = nc.dram_tensor("table0", [cfg.n_nodes, F], bf, kind="ExternalInput")
    t_xfm = nc.dram_tensor("x_fm", [F, p_pad], bf, kind="ExternalInput")
    t_gidx = nc.dram_tensor("gidx", [meta.gidx_total], i16, kind="ExternalInput")
    t_dstloc = nc.dram_tensor("dstloc", [128, meta.nb_total], f32,
                              kind="ExternalInput")
    t_invdeg = nc.dram_tensor("invdeg", [128, cfg.n_win], f32,
                              kind="ExternalInput")
    t_segt = nc.dram_tensor("segt", [cfg.n_win, 128, 128], f32,
                            kind="ExternalInput")
    t_invcnt = nc.dram_tensor("invcnt", [128, 1], f32, kind="ExternalInput")
    t_wfm = nc.dram_tensor("w_fm", [6, F, F], bf, kind="ExternalInput")
    t_bias = nc.dram_tensor("bias_sb", [128, 3], f32, kind="ExternalInput")
    t_wlin = nc.dram_tensor("wlin_fm", [F, cfg.n_cls], f32, kind="ExternalInput")
    t_iota = nc.dram_tensor("iota", [128, 128], bf, kind="ExternalInput")
    t_idbf = nc.dram_tensor("ident_bf", [128, 128], bf, kind="ExternalInput")
    t_idf32 = nc.dram_tensor("ident_f32", [128, 128], f32, kind="ExternalInput")
    t_out = nc.dram_tensor("out_pool", [128, cfg.n_cls], f32,
                           kind="ExternalOutput")

    rg = [list(range(cfg.n_cores))]
    AF = mybir.ActivationFunctionType
    ALU = mybir.AluOpType

    with tile.TileContext(nc) as tc:
        with (
            tc.tile_pool(name="const", bufs=1) as cpool,
            tc.tile_pool(name="xfm", bufs=1) as xpool,
            tc.tile_pool(name="gt", bufs=2) as gpool,
            tc.tile_pool(name="idx", bufs=3) as ipool,
            tc.tile_pool(name="wrk", bufs=4) as wpool,
            tc.tile_pool(name="psA", bufs=3, space="PSUM") as psA,
            tc.tile_pool(name="psT", bufs=2, space="PSUM") as psT,
            tc.tile_pool(name="psH", bufs=2, space="PSUM") as psH,
            tc.tile_pool(name="psP", bufs=1, space="PSUM") as psP,
            tc.tile_pool(name="dram", bufs=1, space="DRAM") as dpool,
        ):
            # ---- constants
            wfm_sb = cpool.tile([128, 6 * F], bf, name="wfm_sb")
            for l in range(6):
                nc.sync.dma_start(wfm_sb[:, l * F:(l + 1) * F], t_wfm.ap()[l])
            bias_sb = cpool.tile([128, 3], f32, name="bias_t")
            nc.sync.dma_start(bias_sb[:], t_bias.ap())
            wlin_sb = cpool.tile([128, cfg.n_cls], f32, name="wlin_sb")
            nc.sync.dma_start(wlin_sb[:], t_wlin.ap())
            iota_sb = cpool.tile([128, 128], bf, name="iota_sb")
            nc.sync.dma_start(iota_sb[:], t_iota.ap())
            idbf_sb = cpool.tile([128, 128], bf, name="idbf_sb")
            nc.sync.dma_start(idbf_sb[:], t_idbf.ap())
            idf32_sb = cpool.tile([128, 128], f32, name="idf32_sb")
            nc.sync.dma_start(idf32_sb[:], t_idf32.ap())
            invdeg_sb = cpool.tile([128, cfg.n_win], f32, name="invdeg_sb")
            nc.sync.dma_start(invdeg_sb[:], t_invdeg.ap())
            invcnt_sb = cpool.tile([128, 1], f32, name="invcnt_sb")
            nc.sync.dma_start(invcnt_sb[:], t_invcnt.ap())
            dstloc_sb = cpool.tile([128, meta.nb_total], f32, name="dstloc_sb")
            nc.sync.dma_start(dstloc_sb[:], t_dstloc.ap())

            x_a = xpool.tile([128, p_pad], bf, name="x_a")
            x_b = xpool.tile([128, p_pad], bf, name="x_b")
            nc.sync.dma_start(x_a[:], t_xfm.ap())

            tabs = [[dpool.tile([cfg.chunk_sizes[k], F], bf,
                                name=f"tab{li}_{k}", addr_space="Shared")
                     for k in range(cfg.n_sub)] for li in range(2)]
            shards = [[dpool.tile([cfg.sub_rows(k), F], bf,
                                  name=f"shard{li}_{k}")
                       for k in range(cfg.n_sub)] for li in range(2)]

            ps_pool = psP.tile([128, 128], f32, name="ps_pool")

            s_const = None
            if skip_s_build:
                s_const = cpool.tile([128, 128], bf, name="s_const")
                nc.vector.memset(s_const[:], 0.0)
            g_const = None
            if skip_gather:
                max_cbs = max(max(r) for r in meta.call_cbs)
                g_const = cpool.tile([128, max_cbs, F], bf, name="g_const")
                nc.vector.memset(g_const[:], 0.0)

            def chunk_src(layer, k):
                if layer == 0:
                    o = cfg.chunk_offs[k]
                    return t_table0.ap()[o:o + cfg.chunk_sizes[k], :]
                return tabs[(layer - 1) % 2][k][:]

            for layer in range(3):
                xin = x_a if layer % 2 == 0 else x_b
                xout = x_b if layer % 2 == 0 else x_a
                for g in range(cfg.n_groups):
                    w0, w1 = cfg.group_wins(g)
                    gtiles = {}
                    for k in range(cfg.n_sub):
                        if (g, k) not in meta.call_of:
                            continue
                        _, off0 = meta.call_of[(g, k)]
                        cbs = meta.call_cbs[g][k]
                        it = ipool.tile([128, cbs * 8], i16, tag=f"idx{k}",
                                        name=f"it{layer}_{g}_{k}")
                        nc.sync.dma_start(
                            it[:],
                            t_gidx.ap()[off0:off0 + cbs * 1024].rearrange(
                                "(p s) -> p s", p=128))
                        gt = gpool.tile([128, cbs, F], bf, tag=f"g{k}",
                                        name=f"gt{layer}_{g}_{k}")
                        if not skip_gather:
                            nc.gpsimd.dma_gather(
                                gt[:], chunk_src(layer, k), it[:],
                                num_idxs=cbs * 128, num_idxs_reg=cbs * 128,
                                elem_size=F, elem_step=F,
                                single_packet=(cbs * 128 <= 1024))
                            gtiles[k] = gt
                        else:
                            gtiles[k] = g_const

                    for w in range(w0, w1):
                        ps_agg = psA.tile([128, F], f32, tag="agg",
                                          name=f"agg{layer}_{w}")
                        nbat = int(meta.bw[w].sum())
                        bi = 0
                        for k in range(cfg.n_sub):
                            for b in range(int(meta.bw[w, k])):
                                col = meta.batch_col[(w, k)] + b
                                cb0 = int(meta.bw[w0:w, k].sum()) + b
                                if skip_s_build:
                                    s_t = s_const
                                else:
                                    s_t = wpool.tile(
                                        [128, 128], bf, tag="S",
                                        name=f"s{layer}_{w}_{k}_{b}")
                                    nc.vector.tensor_scalar(
                                        s_t[:], iota_sb[:],
                                        dstloc_sb[:, col:col + 1], None,
                                        op0=ALU.is_equal)
                                nc.tensor.matmul(
                                    ps_agg[:], s_t[:], gtiles[k][:, cb0, :],
                                    start=(bi == 0), stop=(bi == nbat - 1))
                                bi += 1
                        agg_dm = wpool.tile([128, 128], bf, tag="adm",
                                            name=f"adm{layer}_{w}")
                        nc.vector.tensor_scalar(
                            agg_dm[:], ps_agg[:], invdeg_sb[:, w:w + 1], None,
                            op0=ALU.mult)
                        ps_t = psT.tile([128, 128], bf, tag="pt",
                                        name=f"pt{layer}_{w}")
                        nc.tensor.transpose(ps_t[:], agg_dm[:], idbf_sb[:])
                        agg_fm = wpool.tile([128, 128], bf, tag="afm",
                                            name=f"afm{layer}_{w}")
                        nc.vector.tensor_copy(agg_fm[:], ps_t[:])
                        ps_h = psH.tile([128, 128], f32, tag="ph",
                                        name=f"ph{layer}_{w}")
                        nc.tensor.matmul(
                            ps_h[:], wfm_sb[:, 2 * layer * F:(2 * layer + 1) * F],
                            agg_fm[:], start=True, stop=False)
                        nc.tensor.matmul(
                            ps_h[:],
                            wfm_sb[:, (2 * layer + 1) * F:(2 * layer + 2) * F],
                            xin[:, w * 128:(w + 1) * 128],
                            start=False, stop=True)
                        sub = min(w // cfg.wins_per_sub, cfg.n_sub - 1)
                        wl = w - sub * cfg.wins_per_sub
                        rows = min(cfg.percore - w * 128, 128)
                        if layer < 2:
                            nc.scalar.activation(
                                xout[:, w * 128:(w + 1) * 128], ps_h[:],
                                AF.Relu, bias=bias_sb[:, layer:layer + 1])
                            ps_t2 = psT.tile([128, 128], bf, tag="pt",
                                             name=f"pt2{layer}_{w}")
                            nc.tensor.transpose(
                                ps_t2[:], xout[:, w * 128:(w + 1) * 128],
                                idbf_sb[:])
                            hdm = wpool.tile([128, 128], bf, tag="hdm",
                                             name=f"hdm{layer}_{w}")
                            nc.vector.tensor_copy(hdm[:], ps_t2[:])
                            nc.sync.dma_start(
                                shards[layer][sub][wl * 128:wl * 128 + rows, :],
                                hdm[:rows, :])
                            if not skip_ag and w == min(
                                    (sub + 1) * cfg.wins_per_sub,
                                    cfg.n_win) - 1:
                                nc.gpsimd.collective_compute(
                                    "AllGather", ALU.bypass,
                                    replica_groups=rg,
                                    ins=[shards[layer][sub][:]],
                                    outs=[tabs[layer][sub][:]],
                                )
                        else:
                            h3f = wpool.tile([128, 128], f32, tag="h3f",
                                             name=f"h3f_{w}")
                            nc.scalar.activation(
                                h3f[:], ps_h[:], AF.Identity,
                                bias=bias_sb[:, layer:layer + 1])
                            ps_t2 = psT.tile([128, 128], f32, tag="pt",
                                             name=f"pt2f_{w}")
                            nc.tensor.transpose(ps_t2[:], h3f[:], idf32_sb[:])
                            h3dm = wpool.tile([128, 128], f32, tag="h3dm",
                                              name=f"h3dm_{w}")
                            nc.vector.tensor_copy(h3dm[:], ps_t2[:])
                            segt_t = wpool.tile([128, 128], f32, tag="segt",
                                                name=f"segt_{w}")
                            nc.sync.dma_start(segt_t[:], t_segt.ap()[w])
                            nc.tensor.matmul(
                                ps_pool[:], segt_t[:], h3dm[:],
                                start=(w == 0), stop=(w == cfg.n_win - 1))

            # pooling epilogue + head
            pooled = wpool.tile([128, 128], f32, tag="pooled", name="pooled")
            nc.vector.tensor_scalar(
                pooled[:], ps_pool[:], invcnt_sb[:], None, op0=ALU.mult)
            ps_pf = psT.tile([128, 128], f32, tag="pt", name="ps_pf")
            nc.tensor.transpose(ps_pf[:], pooled[:], idf32_sb[:])
            pooled_fm = wpool.tile([128, 128], f32, tag="pfm", name="pooled_fm")
            nc.vector.tensor_copy(pooled_fm[:], ps_pf[:])
            ps_o = psH.tile([128, cfg.n_cls], f32, tag="ph", name="ps_o")
            nc.tensor.matmul(ps_o[:], pooled_fm[:], wlin_sb[:],
                             start=True, stop=True)
            outt = wpool.tile([128, cfg.n_cls], f32, tag="outt", name="outt")
            nc.vector.tensor_copy(outt[:], ps_o[:])
            nc.sync.dma_start(t_out.ap(), outt[:])

    nc.compile()
    return nc


# ------------------------------------------------------------------------ the entry

def run(cfg: Cfg, inputs: dict, check_with_sim: bool = False, trace: bool = False):
    """Build + run on HW (or CoreSim when check_with_sim); returns (out, results)."""
    params = {k: np.asarray(v) for k, v in inputs.items()
              if k not in ("x", "edge_index", "batch")}
    in_maps, meta, gbase = build_host_data(
        cfg, inputs["x"], inputs["edge_index"], inputs["batch"], params)
    nc = build_program(cfg, meta)

    if check_with_sim:
        from concourse.bass_interp import MultiCoreSim
        sim = MultiCoreSim(nc, num_cores=cfg.n_cores, trace=False)
        for c, core in enumerate(sim.cores.values()):
            for name, arr in in_maps[c].items():
                core.tensor(name)[:] = arr
        sim.simulate(check_with_hw=False)
        results = [{"out_pool": np.asarray(core.tensor("out_pool"))}
                   for core in sim.cores.values()]
        exec_ns = None
    else:
        from concourse.bass_utils import run_bass_kernel_spmd
        res = run_bass_kernel_spmd(nc, in_maps,
                                   core_ids=list(range(cfg.n_cores)),
                                   trace=trace)
        results = res.results
        exec_ns = res.exec_time_ns

    out = np.zeros((cfg.n_graphs, cfg.n_cls), dtype=np.float32)
    for c in range(cfg.n_cores):
        gb = gbase[c]
        hi = min(gb + 128, cfg.n_graphs)
        out[gb:hi] += results[c]["out_pool"][:hi - gb]
    out += np.asarray(inputs["blin"], dtype=np.float32)[None, :]
    return out, exec_ns


def kernel(**inputs) -> np.ndarray:
    out, _ = run(FULL, inputs)
    return out

